# revision 15
# baseline (speedup 1.0000x reference)
"""Trainium2 Bass kernel for Box3dEncoder (nn_Box3dEncoder_75453985456565).

Contract: kernel(**inputs) takes the FULL inputs
    corners3d        [4, 16, 8, 3] f32
    neck_voxel_sizes [4, 3]        f32
and returns the FULL output [4*32768, 2] f32.

Voxel-sharded across 8 cores: each core owns 8 i-rows x 64 j x 8 k x 4 b.

Fast path (no near-vertical box edges, which holds for generic yaw):

  stage 2 (per core, per batch-pair h): telescoped Green's-theorem area.
    With G(u) = relu(u)^2 and the x-grid step equal to the clamp width c,
      iedge(i,j) = [G(ghi_i)-G(ghi_{i+1})] - [G(glo_i)-G(glo_{i+1})]
    since ghi_{i+1} = ghi_i - c.  Six big elementwise passes per h
    ([128, 9, 64]) replace the eleven clamp/square/relu passes:
      ghi, glo (scalar_tensor_tensor), relu x2 (tensor_scalar, 2x DVE mode),
      square x2 (Act), R = PH - PL, R' = R[i]-R[i+1].
    The per-edge dy/(2dx) weight and zov/C rescale (with the first-occurrence
    tie-break epsilon) are folded into fp32r PE matmuls (full-rate at 256
    output cols, numerically exact fp32 in this environment).

  stage 3 (per 128-cell chunk, split by batch-pair so it overlaps h=1
    stage 2): grouped reduce_max over boxes, is_equal one-hot in bf16
    (exact 0/1), PE transpose with a bf16 identity, PSUM->SBUF cast copy,
    and a bf16 selection matmul whose per-box columns are [C_hi, C_lo,
    sin, cos] - C is split into two bf16 halves that accumulate in fp32
    PSUM, preserving the mask-threshold margin.  Threshold mask and the
    interleaved (sin,cos) store follow.

Engine balance: DVE carries tensor_scalar (2x mode) + half the
reductions/equalities, Pool the other half plus one stt per h, Act the
squares, j-chain biases and the PSUM->SBUF copies; all DMAs ride the SP
queue (cheapest HWDGE path).
"""
import numpy as np

B, N, K = 4, 16, 8
CUBE = (64, 64, 8)
LOW = (-32, -32, -4)
NCORES = 8
NI = CUBE[0] // NCORES          # 8 i-rows per core
NJ = CUBE[1]                    # 64
NCELL = NI * NJ                 # 512 cells per core
NCHUNK = NCELL // 128           # 4
V = CUBE[0] * CUBE[1] * CUBE[2]

CW = 128
OFF_Y0, OFF_X0E, OFF_MISC, OFF_COLS = 0, 64, 73, 80

_COMPILED = None


def _host_prep(corners3d, neck_voxel_sizes):
    c = np.asarray(corners3d, np.float32)
    vs = np.asarray(neck_voxel_sizes, np.float32)[0]
    vox_vol = np.float32(vs[0]) * np.float32(vs[1]) * np.float32(vs[2])

    poly = c[:, :, :4, :2]                     # [B,N,4,2]
    nxt = np.roll(poly, -1, axis=2)
    xa, ya = poly[..., 0], poly[..., 1]        # [B,N,4]
    xb, yb = nxt[..., 0], nxt[..., 1]
    dx, dy = xb - xa, yb - ya
    with np.errstate(divide='ignore'):
        inv_dx = np.where(np.abs(dx) < 1e-12, np.float32(0),
                          np.float32(1) / np.where(dx == 0, np.float32(1), dx))
        inv_dy = np.where(np.abs(dy) < 1e-12, np.float32(0),
                          np.float32(1) / np.where(dy == 0, np.float32(1), dy))

    zb0 = c[:, :, :, 2].min(axis=2)
    zb1 = c[:, :, :, 2].max(axis=2)
    quad_area = 0.5 * np.abs((xa * yb - xb * ya).sum(axis=2))
    box_vol = quad_area * (zb1 - zb0)
    C = (vox_vol + box_vol + np.float32(1e-9)).astype(np.float32)   # [B,N]
    invC = (np.float32(1) / C).astype(np.float32)

    kk = np.arange(K, dtype=np.float32) + LOW[2]
    z0 = kk * vs[2]
    z1 = (kk + 1) * vs[2]
    zov = np.maximum(np.minimum(z1[None, :, None], zb1[:, None, :])
                     - np.maximum(z0[None, :, None], zb0[:, None, :]),
                     np.float32(0))                                  # [B,K,N]
    # tie-break epsilon: rho_n scaled by (1+eps_n), eps decreasing in n, so a
    # single reduce_max + is_equal yields the first-occurrence argmax; the C
    # selection weight is divided by (1+eps_n) to compensate exactly.
    eps = (np.float32(15) - np.arange(N, dtype=np.float32)) * np.float32(2.0 ** -20)
    zrho = (zov * invC[:, None, :] * (1 + eps)[None, None, :]).astype(np.float32)
    C_sel = (C / (1 + eps)[None, :]).astype(np.float32)

    d = c[:, :, 0, :2] - c[:, :, 3, :2]
    h = np.sqrt(d[..., 0] ** 2 + d[..., 1] ** 2)
    hs = np.where(h == 0, np.float32(1), h)
    sin = np.where(h > 0, d[..., 1] / hs, np.float32(0)).astype(np.float32)
    cos = np.where(h > 0, d[..., 0] / hs, np.float32(1)).astype(np.float32)

    DX_EPS = np.float32(1e-4)
    vert = np.abs(dx) < DX_EPS
    novert = not bool(vert.any())

    # stage-2 per-partition columns, p = b_lo*64 + n*4 + e, per h-iter
    def colpack(a):    # [B,N,4] -> [2h][128]
        return a.reshape(2, 2, N, 4).reshape(2, 128)
    w1 = (dy * inv_dx * np.float32(0.5))
    cols = np.zeros((2, 128, 8), np.float32)
    cols[:, :, 0] = colpack(inv_dy)
    cols[:, :, 1] = colpack(-ya * inv_dy)
    cols[:, :, 2] = colpack((vs[1] - ya) * inv_dy)
    cols[:, :, 3] = colpack(dx)
    cols[:, :, 4] = colpack(xa)
    cols = np.ascontiguousarray(cols.transpose(1, 0, 2))       # [128,2,8]

    # edge-reduction weights with w1 and zrho folded in:
    # rw[p=(b_lo,n,e), h, (b_lo',k,n')] = (b_lo'==b_lo & n'==n) * w1 * zrho
    w1p = colpack(w1)                          # [2,128]
    rw = np.zeros((128, 2, 2, K, N), np.float32)
    for h2 in range(2):
        for p in range(128):
            b_lo, n = p // 64, (p % 64) // 4
            rw[p, h2, b_lo, :, n] = zrho[2 * h2 + b_lo, :, n] * w1p[h2, p]
    rw = np.ascontiguousarray(rw.reshape(128, 2, 2 * K * N))   # [128,2,256]

    # selection weights (bf16): per b block of 32 cols: (k', q) with
    # q = [C_hi, C_lo, sin, cos]; rows p = (k, n)
    import ml_dtypes
    bf16 = ml_dtypes.bfloat16
    Chi = C_sel.astype(bf16)
    Clo = (C_sel - Chi.astype(np.float32)).astype(bf16)
    w4a = np.zeros((128, B, K, 4), bf16)
    w4b = np.zeros((128, B, K, 4), bf16)
    for p in range(128):
        kq, n = p // N, p % N
        w4a[p, :, kq, 0] = Chi[:, n]
        w4b[p, :, kq, 0] = Clo[:, n]
        w4a[p, :, kq, 2] = sin[:, n].astype(bf16)
        w4a[p, :, kq, 3] = cos[:, n].astype(bf16)
    identbf = np.eye(128, dtype=bf16)
    wib = np.ascontiguousarray(
        np.concatenate([w4a.reshape(128, B * K * 4),
                        w4b.reshape(128, B * K * 4), identbf], axis=1))  # [128,384]

    # cells-major broadcast constants (partition-replicated by host)
    jj = np.arange(NJ, dtype=np.float32) + LOW[1]
    y0 = (jj * vs[1]).astype(np.float32)                       # [64]
    halfvol = np.float32(0.5) * vox_vol

    consts = []
    for m in range(NCORES):
        iiE = np.arange(NI + 1, dtype=np.float32) + (m * NI + LOW[0])
        x0E = (iiE * vs[0]).astype(np.float32)                 # [9]
        row = np.zeros(CW, np.float32)
        row[OFF_Y0:OFF_Y0 + NJ] = y0
        row[OFF_X0E:OFF_X0E + NI + 1] = x0E
        row[OFF_MISC:OFF_MISC + 4] = [halfvol, vs[0], -vs[0], 2 * vs[0]]
        cc = np.broadcast_to(row, (128, CW)).copy()
        cc[:, OFF_COLS:OFF_COLS + 16] = cols.reshape(128, 16)
        consts.append(np.ascontiguousarray(cc))
    return rw, wib, consts, novert


def _build():
    import concourse.bass as bass
    import concourse.tile as tile
    from concourse import bacc, mybir

    f32 = mybir.dt.float32
    f32r = mybir.dt.float32r
    bf16 = mybir.dt.bfloat16
    ALU = mybir.AluOpType
    ACT = mybir.ActivationFunctionType
    X = mybir.AxisListType.X

    nc = bacc.Bacc("TRN2", target_bir_lowering=False, debug=False,
                   num_devices=NCORES)
    d_consts = nc.dram_tensor("consts", [128, CW], f32, kind="ExternalInput")
    d_rw = nc.dram_tensor("rw", [128, 2, 256], f32, kind="ExternalInput")
    d_wib = nc.dram_tensor("wib", [128, 384], bf16, kind="ExternalInput")
    d_out = nc.dram_tensor("out", [B, NCELL * K, 2], f32, kind="ExternalOutput")

    NIE = NI + 1

    with tile.TileContext(nc) as tc:
        with (
            tc.tile_pool(name="const", bufs=1) as cpool,
            tc.tile_pool(name="jc", bufs=2) as jpool,
            tc.tile_pool(name="big", bufs=2) as gpool,
            tc.tile_pool(name="st3", bufs=2) as tpool,
            tc.tile_pool(name="outp", bufs=2) as opool,
            tc.tile_pool(name="rho", bufs=1, space=bass.MemorySpace.PSUM) as rpool,
            tc.tile_pool(name="oht", bufs=2, space=bass.MemorySpace.PSUM) as hpool,
            tc.tile_pool(name="selp", bufs=2, space=bass.MemorySpace.PSUM) as spool,
        ):
            tco = cpool.tile([128, CW], f32, tag="consts")
            nc.sync.dma_start(tco[:], d_consts[:])
            trw = cpool.tile([128, 2, 256], f32, tag="rw")
            nc.sync.dma_start(trw[:], d_rw[:])
            twib = cpool.tile([128, 384], bf16, tag="wib")
            nc.sync.dma_start(twib[:], d_wib[:])

            y0_bc = tco[:, OFF_Y0:OFF_Y0 + NJ]
            x0E_bc = tco[:, OFF_X0E:OFF_X0E + NIE]
            hv_col = tco[:, OFF_MISC:OFF_MISC + 1]

            def col(h, q):
                o = OFF_COLS + h * 8 + q
                return tco[:, o:o + 1]

            def bj(ap):   # [128,64] -> [128,NIE,64]
                return ap[:, None, :].broadcast_to([128, NIE, NJ])

            def bi(ap):   # [128,NIE] -> [128,NIE,64]
                return ap[:, :, None].broadcast_to([128, NIE, NJ])

            # ---- stage 2 + per-h stage 3 ----
            rho_ps = [rpool.tile([128, 2 * 2 * K * N], f32, tag=f"rho{cc}",
                                 name=f"rho{cc}") for cc in range(NCHUNK)]
            maxr = [tpool.tile([128, 2, 2, K], f32, tag=f"mx{cc}",
                               name=f"mx{cc}") for cc in range(NCHUNK)]
            oneh = [tpool.tile([128, 2, 2, K, N], bf16, tag=f"oh{cc}",
                               name=f"oh{cc}") for cc in range(NCHUNK)]
            selsb = [tpool.tile([128, 2, 2, 32], f32, tag=f"ssb{cc}",
                                name=f"ssb{cc}") for cc in range(NCHUNK)]
            outts = [opool.tile([128, B, K, 2], f32, tag=f"outt{cc}",
                                name=f"outt{cc}") for cc in range(NCHUNK)]

            # PE warmup: the clock gate opens only after ~3us of sustained
            # execution, and any idle gap resets the ramp.  A chain of dummy
            # matmuls (into a PSUM range the first real matmul overwrites
            # with start=True) keeps the array busy from t~0.5us so the real
            # matmuls run at the full 2.4 GHz rate.
            wz = cpool.tile([128, 128], f32, tag="wz")
            nc.gpsimd.memset(wz[:], 0.0)
            for _ in range(14):
                nc.tensor.matmul(rho_ps[3][:, 0:128], wz[:], wz[:],
                                 start=True, stop=True)

            for h in range(2):
                # j-chain (small)
                ty0 = jpool.tile([128, NJ], f32, tag="ty0")
                nc.scalar.activation(ty0[:], y0_bc, ACT.Identity,
                                     bias=col(h, 1), scale=col(h, 0))
                ty1 = jpool.tile([128, NJ], f32, tag="ty1")
                nc.scalar.activation(ty1[:], y0_bc, ACT.Identity,
                                     bias=col(h, 2), scale=col(h, 0))
                u0 = jpool.tile([128, NJ], f32, tag="u0")
                nc.vector.tensor_scalar(u0[:], ty0[:], 0.0, 1.0, ALU.max, ALU.min)
                u1 = jpool.tile([128, NJ], f32, tag="u1")
                nc.vector.tensor_scalar(u1[:], ty1[:], 0.0, 1.0, ALU.max, ALU.min)
                lo = jpool.tile([128, NJ], f32, tag="lo")
                nc.vector.tensor_tensor(lo[:], u0[:], u1[:], ALU.min)
                hi = jpool.tile([128, NJ], f32, tag="hi")
                nc.vector.tensor_tensor(hi[:], u0[:], u1[:], ALU.max)
                xme = jpool.tile([128, NIE], f32, tag="xme")
                nc.vector.tensor_single_scalar(xme[:], x0E_bc, col(h, 4),
                                               ALU.subtract)

                # big passes [128, 9, 64]
                ghi = gpool.tile([128, NIE, NJ], f32, tag="ghi")
                nc.vector.scalar_tensor_tensor(ghi[:], bj(hi[:]), col(h, 3),
                                               bi(xme[:]), ALU.mult,
                                               ALU.subtract)
                glo = gpool.tile([128, NIE, NJ], f32, tag="glo")
                nc.vector.scalar_tensor_tensor(glo[:], bj(lo[:]), col(h, 3),
                                               bi(xme[:]), ALU.mult,
                                               ALU.subtract)
                rlh = gpool.tile([128, NIE, NJ], f32, tag="rlh")
                nc.gpsimd.tensor_scalar(rlh[:], ghi[:], 0.0, None, ALU.max)
                rll = gpool.tile([128, NIE, NJ], f32, tag="rll")
                nc.gpsimd.tensor_scalar(rll[:], glo[:], 0.0, None, ALU.max)
                ph = gpool.tile([128, NIE, NJ], f32, tag="ph")
                nc.scalar.activation(ph[:], rlh[:], ACT.Square)
                pl = gpool.tile([128, NIE, NJ], f32, tag="pl")
                nc.scalar.activation(pl[:], rll[:], ACT.Square)
                # R = PH - PL and the i-telescope, split in i-halves so the
                # first chunk's matmul issues before the second half finishes
                rr = gpool.tile([128, NIE, NJ], f32, tag="rr")
                rpr = gpool.tile([128, NI, NJ], f32, tag="rpr")
                for hf in range(2):
                    i0, i1 = 4 * hf, 4 * hf + 5
                    nc.gpsimd.tensor_tensor(rr[:, i0:i1, :], ph[:, i0:i1, :],
                                            pl[:, i0:i1, :], ALU.subtract)
                    nc.vector.tensor_tensor(
                        rpr[:, 4 * hf:4 * hf + 4, :],
                        rr[:, i0:i1 - 1, :], rr[:, i0 + 1:i1, :], ALU.subtract)
                    for cc in (2 * hf, 2 * hf + 1):
                        nc.tensor.matmul(
                            rho_ps[cc][:, h * 256:(h + 1) * 256],
                            rpr[:, 2 * cc:2 * cc + 2, :]
                                .rearrange("p a b -> p (a b)"),
                            trw[:, h, :], start=True, stop=True)

                # ---- stage 3 for this batch-pair over all chunks ----
                for cc in range(NCHUNK):
                    rho_h = rho_ps[cc][:, h * 256:(h + 1) * 256] \
                        .rearrange("p (c k n) -> p c k n", k=K, n=N)
                    mx_h = maxr[cc][:, h, :, :]          # [128, 2, K]
                    nc.vector.tensor_reduce(mx_h, rho_h, X, ALU.max)
                    mx_bc = mx_h[:, :, :, None].broadcast_to([128, 2, K, N])
                    if (cc + h) % 2 == 0:
                        nc.vector.tensor_tensor(oneh[cc][:, h, :, :, :],
                                                rho_h, mx_bc, ALU.is_equal)
                    else:
                        # comparison-free one-hot: relu(1 - BIG*(max-rho)).
                        # Exact for masked cells: their tie-break gaps exceed
                        # 1/BIG; sub-threshold cells are zeroed by the mask.
                        rho_sb = tpool.tile([128, 2, K, N], f32, tag="rsb",
                                            name="rho_sb")
                        nc.scalar.copy(rho_sb[:], rho_h)
                        delta = tpool.tile([128, 2, K, N], f32, tag="dlt",
                                           name="dlt")
                        nc.gpsimd.tensor_tensor(delta[:], mx_bc, rho_sb[:],
                                                ALU.subtract)
                        nc.scalar.activation(oneh[cc][:, h, :, :, :], delta[:],
                                             ACT.Relu, bias=1.0, scale=-4e9)
                    oht = hpool.tile([128, 2, 128], bf16, tag="oht",
                                     name="oht")
                    ohs = tpool.tile([128, 2, 128], bf16, tag="ohs",
                                     name="ohs")
                    sel_ps = spool.tile([128, 2, 32], f32, tag="sps",
                                        name="sps")
                    for bb in range(2):
                        b = 2 * h + bb
                        nc.tensor.transpose(
                            oht[:, bb, :],
                            oneh[cc][:, h, bb, :, :]
                                .rearrange("p k n -> p (k n)"),
                            twib[:, 256:384])
                        nc.scalar.copy(ohs[:, bb, :], oht[:, bb, :])
                        nc.tensor.matmul(
                            sel_ps[:, bb, :], ohs[:, bb, :],
                            twib[:, b * 32:(b + 1) * 32],
                            start=True, stop=False)
                        nc.tensor.matmul(
                            sel_ps[:, bb, :], ohs[:, bb, :],
                            twib[:, 128 + b * 32:128 + (b + 1) * 32],
                            start=False, stop=True)
                    nc.scalar.copy(selsb[cc][:, h, :, :], sel_ps[:])
                if h == 1:
                    for cc in range(NCHUNK):
                        s4 = selsb[cc][:].rearrange(
                            "p g c (k q) -> p (g c) k q", q=4)
                        mxf = maxr[cc][:].rearrange("p g c k -> p (g c) k")
                        isel = tpool.tile([128, B, K], f32, tag="is",
                                          name="is")
                        nc.gpsimd.tensor_tensor(isel[:], s4[:, :, :, 0], mxf,
                                                ALU.mult)
                        mask = tpool.tile([128, B, K], f32, tag="mk",
                                          name="mk")
                        nc.vector.tensor_single_scalar(mask[:], isel[:],
                                                       hv_col, ALU.is_gt)
                        nc.gpsimd.tensor_tensor(
                            outts[cc][:],
                            s4[:, :, :, 2:4],
                            mask[:][:, :, :, None]
                                .broadcast_to([128, B, K, 2]),
                            ALU.mult)
                        nc.sync.dma_start(
                            d_out[:, cc * 128 * K:(cc + 1) * 128 * K, :]
                                .rearrange("b (p k) e -> p b k e", k=K),
                            outts[cc][:])
    nc.compile()
    return nc


def kernel(corners3d, neck_voxel_sizes):
    global _COMPILED
    from concourse.bass_utils import run_bass_kernel_spmd

    rw, wib, consts, novert = _host_prep(corners3d, neck_voxel_sizes)
    assert novert, "near-vertical box edge: fast path inapplicable"
    if _COMPILED is None:
        _COMPILED = (novert, _build())
    nc = _COMPILED[1]
    in_maps = [{"consts": consts[m], "rw": rw, "wib": wib}
               for m in range(NCORES)]
    res = run_bass_kernel_spmd(nc, in_maps, list(range(NCORES)))
    out = np.zeros((B, V, 2), np.float32)
    for m in range(NCORES):
        blk = res.results[m]["out"]          # [B, 4096, 2]
        out[:, m * NCELL * K:(m + 1) * NCELL * K, :] = blk
    return out.reshape(B * V, 2)


# revision 17
# speedup vs baseline: 1.2410x; 1.2410x over previous
"""Trainium2 Bass kernel for Box3dEncoder (nn_Box3dEncoder_75453985456565).

Contract: kernel(**inputs) takes the FULL inputs
    corners3d        [4, 16, 8, 3] f32
    neck_voxel_sizes [4, 3]        f32
and returns the FULL output [4*32768, 2] f32.

Voxel-sharded across 8 cores: each core owns 8 i-rows x 64 j x 8 k x 4 b.

Fast path (no near-vertical box edges, which holds for generic yaw):

  stage 2 (per core, per batch-pair h): telescoped Green's-theorem area.
    With G(u) = relu(u)^2 and the x-grid step equal to the clamp width c,
      iedge(i,j) = [G(ghi_i)-G(ghi_{i+1})] - [G(glo_i)-G(glo_{i+1})]
    since ghi_{i+1} = ghi_i - c.  Six big elementwise passes per h
    ([128, 9, 64]) replace the eleven clamp/square/relu passes:
      ghi, glo (scalar_tensor_tensor), relu x2 (tensor_scalar, 2x DVE mode),
      square x2 (Act), R = PH - PL, R' = R[i]-R[i+1].
    The per-edge dy/(2dx) weight and zov/C rescale (with the first-occurrence
    tie-break epsilon) are folded into fp32r PE matmuls (full-rate at 256
    output cols, numerically exact fp32 in this environment).

  stage 3 (per 128-cell chunk, split by batch-pair so it overlaps h=1
    stage 2): grouped reduce_max over boxes, is_equal one-hot in bf16
    (exact 0/1), PE transpose with a bf16 identity, PSUM->SBUF cast copy,
    and a bf16 selection matmul whose per-box columns are [C_hi, C_lo,
    sin, cos] - C is split into two bf16 halves that accumulate in fp32
    PSUM, preserving the mask-threshold margin.  Threshold mask and the
    interleaved (sin,cos) store follow.

Engine balance: DVE carries tensor_scalar (2x mode) + half the
reductions/equalities, Pool the other half plus one stt per h, Act the
squares, j-chain biases and the PSUM->SBUF copies; all DMAs ride the SP
queue (cheapest HWDGE path).
"""
import numpy as np

B, N, K = 4, 16, 8
CUBE = (64, 64, 8)
LOW = (-32, -32, -4)
NCORES = 8
NI = CUBE[0] // NCORES          # 8 i-rows per core
NJ = CUBE[1]                    # 64
NCELL = NI * NJ                 # 512 cells per core
NCHUNK = NCELL // 128           # 4
V = CUBE[0] * CUBE[1] * CUBE[2]

CW = 128
OFF_Y0, OFF_X0E, OFF_MISC, OFF_COLS = 0, 64, 73, 80

_COMPILED = None


def _host_prep(corners3d, neck_voxel_sizes):
    c = np.asarray(corners3d, np.float32)
    vs = np.asarray(neck_voxel_sizes, np.float32)[0]
    vox_vol = np.float32(vs[0]) * np.float32(vs[1]) * np.float32(vs[2])

    poly = c[:, :, :4, :2]                     # [B,N,4,2]
    nxt = np.roll(poly, -1, axis=2)
    xa, ya = poly[..., 0], poly[..., 1]        # [B,N,4]
    xb, yb = nxt[..., 0], nxt[..., 1]
    dx, dy = xb - xa, yb - ya
    with np.errstate(divide='ignore'):
        inv_dx = np.where(np.abs(dx) < 1e-12, np.float32(0),
                          np.float32(1) / np.where(dx == 0, np.float32(1), dx))
        inv_dy = np.where(np.abs(dy) < 1e-12, np.float32(0),
                          np.float32(1) / np.where(dy == 0, np.float32(1), dy))

    zb0 = c[:, :, :, 2].min(axis=2)
    zb1 = c[:, :, :, 2].max(axis=2)
    quad_area = 0.5 * np.abs((xa * yb - xb * ya).sum(axis=2))
    box_vol = quad_area * (zb1 - zb0)
    C = (vox_vol + box_vol + np.float32(1e-9)).astype(np.float32)   # [B,N]
    invC = (np.float32(1) / C).astype(np.float32)

    kk = np.arange(K, dtype=np.float32) + LOW[2]
    z0 = kk * vs[2]
    z1 = (kk + 1) * vs[2]
    zov = np.maximum(np.minimum(z1[None, :, None], zb1[:, None, :])
                     - np.maximum(z0[None, :, None], zb0[:, None, :]),
                     np.float32(0))                                  # [B,K,N]
    # tie-break epsilon: rho_n scaled by (1+eps_n), eps decreasing in n, so a
    # single reduce_max + is_equal yields the first-occurrence argmax; the C
    # selection weight is divided by (1+eps_n) to compensate exactly.
    eps = (np.float32(15) - np.arange(N, dtype=np.float32)) * np.float32(2.0 ** -20)
    zrho = (zov * invC[:, None, :] * (1 + eps)[None, None, :]).astype(np.float32)
    C_sel = (C / (1 + eps)[None, :]).astype(np.float32)

    d = c[:, :, 0, :2] - c[:, :, 3, :2]
    h = np.sqrt(d[..., 0] ** 2 + d[..., 1] ** 2)
    hs = np.where(h == 0, np.float32(1), h)
    sin = np.where(h > 0, d[..., 1] / hs, np.float32(0)).astype(np.float32)
    cos = np.where(h > 0, d[..., 0] / hs, np.float32(1)).astype(np.float32)

    DX_EPS = np.float32(1e-4)
    vert = np.abs(dx) < DX_EPS
    novert = not bool(vert.any())

    # stage-2 per-partition columns, p = b_lo*64 + n*4 + e, per h-iter
    def colpack(a):    # [B,N,4] -> [2h][128]
        return a.reshape(2, 2, N, 4).reshape(2, 128)
    w1 = (dy * inv_dx * np.float32(0.5))
    cols = np.zeros((2, 128, 8), np.float32)
    cols[:, :, 0] = colpack(inv_dy)
    cols[:, :, 1] = colpack(-ya * inv_dy)
    cols[:, :, 2] = colpack((vs[1] - ya) * inv_dy)
    cols[:, :, 3] = colpack(dx)
    cols[:, :, 4] = colpack(xa)
    cols = np.ascontiguousarray(cols.transpose(1, 0, 2))       # [128,2,8]

    # edge-reduction weights with w1 and zrho folded in:
    # rw[p=(b_lo,n,e), h, (b_lo',k,n')] = (b_lo'==b_lo & n'==n) * w1 * zrho
    w1p = colpack(w1)                          # [2,128]
    rw = np.zeros((128, 2, 2, K, N), np.float32)
    for h2 in range(2):
        for p in range(128):
            b_lo, n = p // 64, (p % 64) // 4
            rw[p, h2, b_lo, :, n] = zrho[2 * h2 + b_lo, :, n] * w1p[h2, p]
    rw = np.ascontiguousarray(rw.reshape(128, 2, 2 * K * N))   # [128,2,256]

    # selection weights (bf16): per b block of 32 cols: (k', q) with
    # q = [C_hi, C_lo, sin, cos]; rows p = (k, n)
    import ml_dtypes
    bf16 = ml_dtypes.bfloat16
    Chi = C_sel.astype(bf16)
    Clo = (C_sel - Chi.astype(np.float32)).astype(bf16)
    w4a = np.zeros((128, B, K, 4), bf16)
    w4b = np.zeros((128, B, K, 4), bf16)
    for p in range(128):
        kq, n = p // N, p % N
        w4a[p, :, kq, 0] = Chi[:, n]
        w4b[p, :, kq, 0] = Clo[:, n]
        w4a[p, :, kq, 2] = sin[:, n].astype(bf16)
        w4a[p, :, kq, 3] = cos[:, n].astype(bf16)
    identbf = np.eye(128, dtype=bf16)
    wib = np.ascontiguousarray(
        np.concatenate([w4a.reshape(128, B * K * 4),
                        w4b.reshape(128, B * K * 4), identbf], axis=1))  # [128,384]

    # cells-major broadcast constants (partition-replicated by host)
    jj = np.arange(NJ, dtype=np.float32) + LOW[1]
    y0 = (jj * vs[1]).astype(np.float32)                       # [64]
    halfvol = np.float32(0.5) * vox_vol

    consts = []
    for m in range(NCORES):
        iiE = np.arange(NI + 1, dtype=np.float32) + (m * NI + LOW[0])
        x0E = (iiE * vs[0]).astype(np.float32)                 # [9]
        row = np.zeros(CW, np.float32)
        row[OFF_Y0:OFF_Y0 + NJ] = y0
        row[OFF_X0E:OFF_X0E + NI + 1] = x0E
        row[OFF_MISC:OFF_MISC + 4] = [halfvol, vs[0], -vs[0], 2 * vs[0]]
        cc = np.broadcast_to(row, (128, CW)).copy()
        cc[:, OFF_COLS:OFF_COLS + 16] = cols.reshape(128, 16)
        consts.append(np.ascontiguousarray(cc))
    return rw, wib, consts, novert


def _build():
    import concourse.bass as bass
    import concourse.tile as tile
    from concourse import bacc, mybir

    f32 = mybir.dt.float32
    f32r = mybir.dt.float32r
    bf16 = mybir.dt.bfloat16
    ALU = mybir.AluOpType
    ACT = mybir.ActivationFunctionType
    X = mybir.AxisListType.X

    nc = bacc.Bacc("TRN2", target_bir_lowering=False, debug=False,
                   num_devices=NCORES)
    d_consts = nc.dram_tensor("consts", [128, CW], f32, kind="ExternalInput")
    d_rw = nc.dram_tensor("rw", [128, 2, 256], f32, kind="ExternalInput")
    d_wib = nc.dram_tensor("wib", [128, 384], bf16, kind="ExternalInput")
    d_out = nc.dram_tensor("out", [B, NCELL * K, 2], f32, kind="ExternalOutput")

    NIE = NI + 1

    with tile.TileContext(nc) as tc:
        with (
            tc.tile_pool(name="const", bufs=1) as cpool,
            tc.tile_pool(name="jc", bufs=2) as jpool,
            tc.tile_pool(name="big", bufs=2) as gpool,
            tc.tile_pool(name="st3", bufs=2) as tpool,
            tc.tile_pool(name="outp", bufs=2) as opool,
            tc.tile_pool(name="rho", bufs=1, space=bass.MemorySpace.PSUM) as rpool,
            tc.tile_pool(name="oht", bufs=2, space=bass.MemorySpace.PSUM) as hpool,
            tc.tile_pool(name="selp", bufs=2, space=bass.MemorySpace.PSUM) as spool,
        ):
            tco = cpool.tile([128, CW], f32, tag="consts")
            nc.sync.dma_start(tco[:], d_consts[:])
            trw = cpool.tile([128, 2, 256], f32, tag="rw")
            nc.sync.dma_start(trw[:], d_rw[:])
            twib = cpool.tile([128, 384], bf16, tag="wib")
            nc.sync.dma_start(twib[:], d_wib[:])

            y0_bc = tco[:, OFF_Y0:OFF_Y0 + NJ]
            x0E_bc = tco[:, OFF_X0E:OFF_X0E + NIE]
            hv_col = tco[:, OFF_MISC:OFF_MISC + 1]

            def col(h, q):
                o = OFF_COLS + h * 8 + q
                return tco[:, o:o + 1]

            def bj(ap):   # [128,64] -> [128,NIE,64]
                return ap[:, None, :].broadcast_to([128, NIE, NJ])

            def bi(ap):   # [128,NIE] -> [128,NIE,64]
                return ap[:, :, None].broadcast_to([128, NIE, NJ])

            # ---- stage 2 + per-h stage 3 ----
            rho_ps = [rpool.tile([128, 2 * 2 * K * N], f32, tag=f"rho{cc}",
                                 name=f"rho{cc}") for cc in range(NCHUNK)]
            maxr = [tpool.tile([128, 2, 2, K], f32, tag=f"mx{cc}",
                               name=f"mx{cc}") for cc in range(NCHUNK)]
            oneh = [tpool.tile([128, 2, 2, K, N], bf16, tag=f"oh{cc}",
                               name=f"oh{cc}") for cc in range(NCHUNK)]
            selsb = [tpool.tile([128, 2, 2, 32], f32, tag=f"ssb{cc}",
                                name=f"ssb{cc}") for cc in range(NCHUNK)]
            outts = [opool.tile([128, B, K, 2], f32, tag=f"outt{cc}",
                                name=f"outt{cc}") for cc in range(NCHUNK)]

            # PE warmup: the clock gate opens only after ~3us of sustained
            # execution, and any idle gap resets the ramp.  A chain of dummy
            # matmuls (into a PSUM range the first real matmul overwrites
            # with start=True) keeps the array busy from t~0.5us so the real
            # matmuls run at the full 2.4 GHz rate.
            wz = cpool.tile([128, 128], f32, tag="wz")
            nc.gpsimd.memset(wz[:], 0.0)
            for _ in range(14):
                nc.tensor.matmul(rho_ps[3][:, 0:128], wz[:], wz[:],
                                 start=True, stop=True)

            for h in range(2):
                # j-chain (small)
                ty0 = jpool.tile([128, NJ], f32, tag="ty0")
                nc.scalar.activation(ty0[:], y0_bc, ACT.Identity,
                                     bias=col(h, 1), scale=col(h, 0))
                ty1 = jpool.tile([128, NJ], f32, tag="ty1")
                nc.scalar.activation(ty1[:], y0_bc, ACT.Identity,
                                     bias=col(h, 2), scale=col(h, 0))
                u0 = jpool.tile([128, NJ], f32, tag="u0")
                nc.gpsimd.tensor_scalar(u0[:], ty0[:], 0.0, 1.0, ALU.max, ALU.min)
                u1 = jpool.tile([128, NJ], f32, tag="u1")
                nc.gpsimd.tensor_scalar(u1[:], ty1[:], 0.0, 1.0, ALU.max, ALU.min)
                lo = jpool.tile([128, NJ], f32, tag="lo")
                nc.vector.tensor_tensor(lo[:], u0[:], u1[:], ALU.min)
                hi = jpool.tile([128, NJ], f32, tag="hi")
                nc.vector.tensor_tensor(hi[:], u0[:], u1[:], ALU.max)
                xme = jpool.tile([128, NIE], f32, tag="xme")
                nc.vector.tensor_single_scalar(xme[:], x0E_bc, col(h, 4),
                                               ALU.subtract)

                # big passes [128, 9, 64]
                ghi = gpool.tile([128, NIE, NJ], f32, tag="ghi")
                nc.vector.scalar_tensor_tensor(ghi[:], bj(hi[:]), col(h, 3),
                                               bi(xme[:]), ALU.mult,
                                               ALU.subtract)
                glo = gpool.tile([128, NIE, NJ], f32, tag="glo")
                nc.vector.scalar_tensor_tensor(glo[:], bj(lo[:]), col(h, 3),
                                               bi(xme[:]), ALU.mult,
                                               ALU.subtract)
                rlh = gpool.tile([128, NIE, NJ], f32, tag="rlh")
                nc.vector.tensor_scalar(rlh[:], ghi[:], 0.0, None, ALU.max)
                rll = gpool.tile([128, NIE, NJ], f32, tag="rll")
                nc.vector.tensor_scalar(rll[:], glo[:], 0.0, None, ALU.max)
                ph = gpool.tile([128, NIE, NJ], f32, tag="ph")
                nc.scalar.activation(ph[:], rlh[:], ACT.Square)
                pl = gpool.tile([128, NIE, NJ], f32, tag="pl")
                nc.scalar.activation(pl[:], rll[:], ACT.Square)
                # R = PH - PL and the i-telescope, split in i-halves so the
                # first chunk's matmul issues before the second half finishes
                rr = gpool.tile([128, NIE, NJ], f32, tag="rr")
                rpr = gpool.tile([128, NI, NJ], f32, tag="rpr")
                for hf in range(2):
                    i0, i1 = 4 * hf, 4 * hf + 5
                    nc.gpsimd.tensor_tensor(rr[:, i0:i1, :], ph[:, i0:i1, :],
                                            pl[:, i0:i1, :], ALU.subtract)
                    nc.gpsimd.tensor_tensor(
                        rpr[:, 4 * hf:4 * hf + 4, :],
                        rr[:, i0:i1 - 1, :], rr[:, i0 + 1:i1, :], ALU.subtract)
                    for cc in (2 * hf, 2 * hf + 1):
                        nc.tensor.matmul(
                            rho_ps[cc][:, h * 256:(h + 1) * 256],
                            rpr[:, 2 * cc:2 * cc + 2, :]
                                .rearrange("p a b -> p (a b)"),
                            trw[:, h, :], start=True, stop=True)

                # ---- stage 3 for this batch-pair over all chunks ----
                for cc in range(NCHUNK):
                    rho_h = rho_ps[cc][:, h * 256:(h + 1) * 256] \
                        .rearrange("p (c k n) -> p c k n", k=K, n=N)
                    mx_h = maxr[cc][:, h, :, :]          # [128, 2, K]
                    nc.vector.tensor_reduce(mx_h, rho_h, X, ALU.max)
                    mx_bc = mx_h[:, :, :, None].broadcast_to([128, 2, K, N])
                    if not (h == 1 and cc in (1, 3)):
                        nc.vector.tensor_tensor(oneh[cc][:, h, :, :, :],
                                                rho_h, mx_bc, ALU.is_equal)
                    else:
                        # comparison-free one-hot: relu(1 - BIG*(max-rho)).
                        # Exact for masked cells: their tie-break gaps exceed
                        # 1/BIG; sub-threshold cells are zeroed by the mask.
                        rho_sb = tpool.tile([128, 2, K, N], f32, tag="rsb",
                                            name="rho_sb")
                        nc.scalar.copy(rho_sb[:], rho_h)
                        delta = tpool.tile([128, 2, K, N], f32, tag="dlt",
                                           name="dlt")
                        nc.gpsimd.tensor_tensor(delta[:], mx_bc, rho_sb[:],
                                                ALU.subtract)
                        nc.scalar.activation(oneh[cc][:, h, :, :, :], delta[:],
                                             ACT.Relu, bias=1.0, scale=-4e9)
                    oht = hpool.tile([128, 2, 128], bf16, tag="oht",
                                     name="oht")
                    ohs = tpool.tile([128, 2, 128], bf16, tag="ohs",
                                     name="ohs")
                    sel_ps = spool.tile([128, 2, 32], f32, tag="sps",
                                        name="sps")
                    for bb in range(2):
                        nc.tensor.transpose(
                            oht[:, bb, :],
                            oneh[cc][:, h, bb, :, :]
                                .rearrange("p k n -> p (k n)"),
                            twib[:, 256:384])
                    nc.scalar.copy(ohs[:], oht[:])
                    for bb in range(2):
                        b = 2 * h + bb
                        nc.tensor.matmul(
                            sel_ps[:, bb, :], ohs[:, bb, :],
                            twib[:, b * 32:(b + 1) * 32],
                            start=True, stop=False)
                        nc.tensor.matmul(
                            sel_ps[:, bb, :], ohs[:, bb, :],
                            twib[:, 128 + b * 32:128 + (b + 1) * 32],
                            start=False, stop=True)
                    nc.scalar.copy(selsb[cc][:, h, :, :], sel_ps[:])
                if h == 1:
                    for cc in range(NCHUNK):
                        s4 = selsb[cc][:].rearrange(
                            "p g c (k q) -> p (g c) k q", q=4)
                        mxf = maxr[cc][:].rearrange("p g c k -> p (g c) k")
                        isel = tpool.tile([128, B, K], f32, tag="is",
                                          name="is")
                        nc.gpsimd.tensor_tensor(isel[:], s4[:, :, :, 0], mxf,
                                                ALU.mult)
                        mask = tpool.tile([128, B, K], f32, tag="mk",
                                          name="mk")
                        nc.vector.tensor_single_scalar(mask[:], isel[:],
                                                       hv_col, ALU.is_gt)
                        nc.gpsimd.tensor_tensor(
                            outts[cc][:],
                            s4[:, :, :, 2:4],
                            mask[:][:, :, :, None]
                                .broadcast_to([128, B, K, 2]),
                            ALU.mult)
                        nc.sync.dma_start(
                            d_out[:, cc * 128 * K:(cc + 1) * 128 * K, :]
                                .rearrange("b (p k) e -> p b k e", k=K),
                            outts[cc][:])
    nc.compile()
    return nc


def kernel(corners3d, neck_voxel_sizes):
    global _COMPILED
    from concourse.bass_utils import run_bass_kernel_spmd

    rw, wib, consts, novert = _host_prep(corners3d, neck_voxel_sizes)
    assert novert, "near-vertical box edge: fast path inapplicable"
    if _COMPILED is None:
        _COMPILED = (novert, _build())
    nc = _COMPILED[1]
    in_maps = [{"consts": consts[m], "rw": rw, "wib": wib}
               for m in range(NCORES)]
    res = run_bass_kernel_spmd(nc, in_maps, list(range(NCORES)))
    out = np.zeros((B, V, 2), np.float32)
    for m in range(NCORES):
        blk = res.results[m]["out"]          # [B, 4096, 2]
        out[:, m * NCELL * K:(m + 1) * NCELL * K, :] = blk
    return out.reshape(B * V, 2)
